# revision 21
# baseline (speedup 1.0000x reference)
"""Distributed Trainium2 kernel for single-head attention with QKV projections.

Reference computation (B=4, N=4096, D=256, fp32):
    q = x @ Wq_w.T + Wq_b
    k = z @ Wk_w.T + Wk_b
    v = z @ Wv_w.T + Wv_b
    out = softmax(q @ k.T / sqrt(D)) @ v

Sharding: pure data-parallel over (batch, query-half) across 8 cores. Core c
handles batch b = c//2, query rows [h*2048, (h+1)*2048) with h = c%2, and holds
the full z[b] so K/V are recomputed per core (2x duplication of the tiny D*D
projections). No collectives.

Layout: everything is kept "transposed" so no PE transposes are needed:
  - qT[e, i] / kT[e, j] come out of the projection matmuls with the feature dim
    on partitions, which is exactly the lhsT/rhs layout the scores matmul wants.
  - scores are computed transposed, sT[j, i] (keys on partitions), so the exp'd
    probabilities are directly the lhsT of the PV matmul.
  - a ones-column appended to v makes the PV matmul also produce the softmax
    denominator; normalization + Wv_b bias-add are fused into one DVE op.
Compute is bf16 (PE at 1 cycle/row vs 4 for fp32), accumulation fp32 in PSUM.
Softmax skips max-subtraction: scores/sqrt(D) are ~N(0, 0.65) here, bounded
by ~+-4, so exp() is safe in fp32.

Constants are packed host-side into two [128, *] DRAM tensors (Wpack/Bpack):
DMA issue cost on the sequencer is ~5ns per descriptor ~= per partition-row,
so one wide transfer beats many narrow ones.
"""

import numpy as np
import ml_dtypes

B, N, D = 4, 4096, 256
NCORES = 8
S = N // 2          # query rows per core
P = 128             # partitions
QBLK = 512          # scores free-dim block (one PSUM bank)
NJT = N // P        # 32 key tiles
NQB = S // QBLK     # 4 query blocks per core
DC = D // P         # 2 chunks of the feature dim
NWARM = 22          # PE p-state warm-up matmuls (bridge until first input DMA)

BF16 = ml_dtypes.bfloat16

_CACHE = {}


def _build():
    import concourse.mybir as mybir
    import concourse.tile as tile
    from concourse import bacc

    bf16 = mybir.dt.bfloat16
    f32 = mybir.dt.float32
    AF = mybir.ActivationFunctionType
    ALU = mybir.AluOpType

    nc = bacc.Bacc("TRN2", target_bir_lowering=False, debug=False, num_devices=NCORES)

    xT = nc.dram_tensor("xT", [DC, P, S], bf16, kind="ExternalInput").ap()
    zT = nc.dram_tensor("zT", [DC, P, N], bf16, kind="ExternalInput").ap()
    Wpack = nc.dram_tensor("Wpack", [P, 6 * D], bf16, kind="ExternalInput").ap()
    Bpack = nc.dram_tensor("Bpack", [P, 6 + D], f32, kind="ExternalInput").ap()
    out = nc.dram_tensor("out", [S, D], f32, kind="ExternalOutput").ap()

    with tile.TileContext(nc) as tc:
        with (
            tc.tile_pool(name="consts", bufs=1) as cp,
            tc.tile_pool(name="big", bufs=1) as bp,
            tc.tile_pool(name="pblk", bufs=2) as pp,
            tc.tile_pool(name="outp", bufs=4) as op,
            tc.tile_pool(name="psum", bufs=4, space="PSUM") as ps,
        ):
            # ---- PE warm-up ----
            # The PE clock ramps 0.65 -> 1.2 -> 2.4 GHz over ~4us of
            # continuous execution and resets on idle. Burn a few matmuls on
            # an uninitialized tile while the input DMAs are in flight so the
            # real matmuls start (nearly) at full clock.
            wrm = cp.tile([P, P + QBLK], bf16, tag="warm", name="warm")
            nc.vector.memset(wrm[:], 0.0)
            wps = ps.tile([P, QBLK], f32, tag="proj", name="warm_ps")
            for _ in range(NWARM):
                nc.tensor.matmul(
                    wps[:], wrm[:, 0:P], wrm[:, P:P + QBLK], start=True, stop=True
                )

            # ---- input / constant DMAs (ordered by first use) ----
            # Two HW-DGE rings issue in parallel: sync (SP) carries the
            # weights + zT, scalar (ACT) carries xT + biases. ~0.6us issue
            # cost per [128, *] transfer (one descriptor per partition row).
            wpk = cp.tile([P, 6 * D], bf16, tag="wpk", name="wpk")
            nc.sync.dma_start(wpk[:], Wpack[:])
            xT_sb, zT_sb = [], []
            for c in range(DC):
                xT_sb.append(bp.tile([P, S], bf16, tag=f"xT{c}", name=f"xT{c}"))
                zT_sb.append(bp.tile([P, N], bf16, tag=f"zT{c}", name=f"zT{c}"))
            for c in range(DC):
                nc.sync.dma_start(zT_sb[c][:], zT[c])
            for c in range(DC):
                nc.sync.dma_start(xT_sb[c][:], xT[c])
            bpk = cp.tile([P, 6 + D], f32, tag="bpk", name="bpk")
            nc.sync.dma_start(bpk[:], Bpack[:])

            def w_sl(w, c):  # lhsT [128, 256] slice for weight w, chunk c
                return wpk[:, (w * 2 + c) * D:(w * 2 + c + 1) * D]

            bq_sb = [bpk[:, c:c + 1] for c in range(DC)]
            bk_sb = [bpk[:, 2 + c:3 + c] for c in range(DC)]
            bvb_sb = bpk[:, 6:6 + D]

            # ---- kT projection: kT[e, j] over [256, 4096] ----
            kT_sb = []
            for e in range(DC):
                t = bp.tile([P, N], bf16, tag=f"kT{e}", name=f"kT{e}")
                kT_sb.append(t)
            for jb in range(N // QBLK):
                for e in range(DC):
                    acc = ps.tile([P, QBLK], f32, tag="proj", name="proj_ps")
                    for c in range(DC):
                        nc.tensor.matmul(
                            acc[:],
                            w_sl(1, c)[:, e * P:(e + 1) * P],
                            zT_sb[c][:, jb * QBLK:(jb + 1) * QBLK],
                            start=(c == 0),
                            stop=(c == DC - 1),
                        )
                    nc.scalar.activation(
                        kT_sb[e][:, jb * QBLK:(jb + 1) * QBLK], acc[:],
                        AF.Identity, bias=bk_sb[e],
                    )

            # ---- v projection: v[j, e] in 32 tiles [128, 257] (ones column) ----
            v_sb = []
            for t_i in range(NJT):
                vt = bp.tile([P, D + 1], bf16, tag=f"v{t_i}", name=f"v{t_i}")
                acc = ps.tile([P, D], f32, tag="sc", name="v_ps")
                for c in range(DC):
                    nc.tensor.matmul(
                        acc[:],
                        zT_sb[c][:, t_i * P:(t_i + 1) * P],
                        w_sl(2, c),
                        start=(c == 0),
                        stop=(c == DC - 1),
                    )
                nc.vector.tensor_copy(vt[:, 0:D], acc[:])
                nc.vector.memset(vt[:, D:D + 1], 1.0)
                v_sb.append(vt)

            # ---- qT projection: qT[e, i] over [256, 2048] ----
            qT_sb = []
            for e in range(DC):
                t = bp.tile([P, S], bf16, tag=f"qT{e}", name=f"qT{e}")
                qT_sb.append(t)
            for jb in range(S // QBLK):
                for e in range(DC):
                    acc = ps.tile([P, QBLK], f32, tag="proj", name="proj_ps")
                    for c in range(DC):
                        nc.tensor.matmul(
                            acc[:],
                            w_sl(0, c)[:, e * P:(e + 1) * P],
                            xT_sb[c][:, jb * QBLK:(jb + 1) * QBLK],
                            start=(c == 0),
                            stop=(c == DC - 1),
                        )
                    nc.scalar.activation(
                        qT_sb[e][:, jb * QBLK:(jb + 1) * QBLK], acc[:],
                        AF.Identity, bias=bq_sb[e],
                    )


            # ---- attention, per query block of 512 ----
            for qb in range(NQB):
                pts = []
                for t_i in range(NJT):
                    acc = ps.tile([P, QBLK], f32, tag="sc", name="sc_ps")
                    for c in range(DC):
                        nc.tensor.matmul(
                            acc[:],
                            kT_sb[c][:, t_i * P:(t_i + 1) * P],
                            qT_sb[c][:, qb * QBLK:(qb + 1) * QBLK],
                            start=(c == 0),
                            stop=(c == DC - 1),
                        )
                    pt = pp.tile([P, QBLK], bf16, tag=f"pT{t_i}", name=f"pT{t_i}")
                    # exp(scores / sqrt(D)); sqrt(256) = 16
                    nc.scalar.activation(pt[:], acc[:], AF.Exp, scale=1.0 / 16.0)
                    pts.append(pt)
                # PV accumulation with t outer so PE consumes exp'd tiles in
                # the order ACT produces them (no stall on the last exps).
                pvs = [
                    ps.tile([P, D + 1], f32, tag="proj", name=f"pv_ps{sq}")
                    for sq in range(QBLK // P)
                ]
                for t_i in range(NJT):
                    for sq in range(QBLK // P):
                        nc.tensor.matmul(
                            pvs[sq][:],
                            pts[t_i][:, sq * P:(sq + 1) * P],
                            v_sb[t_i][:],
                            start=(t_i == 0),
                            stop=(t_i == NJT - 1),
                        )
                for sq in range(QBLK // P):
                    pv = pvs[sq]
                    recip = op.tile([P, 1], f32, tag="recip", name="recip")
                    nc.vector.reciprocal(recip[:], pv[:, D:D + 1])
                    ot = op.tile([P, D], f32, tag="ot", name="ot")
                    # out = (pv * 1/denom) + bv
                    nc.vector.scalar_tensor_tensor(
                        ot[:], pv[:, 0:D], recip[:], bvb_sb,
                        op0=ALU.mult, op1=ALU.add,
                    )
                    r0 = (qb * (QBLK // P) + sq) * P
                    # alternate DGE rings so the last output DMAs issue in
                    # parallel instead of serializing on one sequencer
                    eng = nc.scalar if sq % 2 else nc.sync
                    eng.dma_start(out[r0:r0 + P, :], ot[:])

    nc.compile()
    return nc


def _get_nc():
    if "nc" not in _CACHE:
        _CACHE["nc"] = _build()
    return _CACHE["nc"]


def _prep_in_maps(x, z, Wq_w, Wq_b, Wk_w, Wk_b, Wv_w, Wv_b):
    x = np.asarray(x, np.float32)
    z = np.asarray(z, np.float32)

    Wpack = np.empty((P, 6 * D), BF16)
    for w, W in enumerate((Wq_w, Wk_w, Wv_w)):
        WT = np.ascontiguousarray(np.asarray(W, np.float32).T).astype(BF16)
        for c in range(DC):
            Wpack[:, (w * 2 + c) * D:(w * 2 + c + 1) * D] = WT[c * P:(c + 1) * P, :]
    Bpack = np.zeros((P, 6 + D), np.float32)
    for c in range(DC):
        Bpack[:, c] = np.asarray(Wq_b, np.float32)[c * P:(c + 1) * P]
        Bpack[:, 2 + c] = np.asarray(Wk_b, np.float32)[c * P:(c + 1) * P]
    Bpack[:, 6:] = np.broadcast_to(np.asarray(Wv_b, np.float32), (P, D))

    in_maps = []
    for core in range(NCORES):
        b, h = divmod(core, 2)
        xTc = np.ascontiguousarray(x[b].T[:, h * S:(h + 1) * S]).astype(BF16)
        zTc = np.ascontiguousarray(z[b].T).astype(BF16)
        in_maps.append({
            "xT": xTc.reshape(DC, P, S),
            "zT": zTc.reshape(DC, P, N),
            "Wpack": Wpack, "Bpack": Bpack,
        })
    return in_maps


def kernel(x, z, Wq_w, Wq_b, Wk_w, Wk_b, Wv_w, Wv_b):
    from concourse.bass_utils import run_bass_kernel_spmd

    in_maps = _prep_in_maps(x, z, Wq_w, Wq_b, Wk_w, Wk_b, Wv_w, Wv_b)
    nc = _get_nc()
    _CACHE["in_maps"] = in_maps
    res = run_bass_kernel_spmd(nc, in_maps, core_ids=list(range(NCORES)))

    full = np.empty((B, N, D), np.float32)
    for core in range(NCORES):
        b, h = divmod(core, 2)
        full[b, h * S:(h + 1) * S, :] = res.results[core]["out"]
    return full


# revision 22
# speedup vs baseline: 1.0095x; 1.0095x over previous
"""Distributed Trainium2 kernel for single-head attention with QKV projections.

Reference computation (B=4, N=4096, D=256, fp32):
    q = x @ Wq_w.T + Wq_b
    k = z @ Wk_w.T + Wk_b
    v = z @ Wv_w.T + Wv_b
    out = softmax(q @ k.T / sqrt(D)) @ v

Sharding: pure data-parallel over (batch, query-half) across 8 cores. Core c
handles batch b = c//2, query rows [h*2048, (h+1)*2048) with h = c%2, and holds
the full z[b] so K/V are recomputed per core (2x duplication of the tiny D*D
projections). No collectives.

Layout: everything is kept "transposed" so no PE transposes are needed:
  - qT[e, i] / kT[e, j] come out of the projection matmuls with the feature dim
    on partitions, which is exactly the lhsT/rhs layout the scores matmul wants.
  - scores are computed transposed, sT[j, i] (keys on partitions), so the exp'd
    probabilities are directly the lhsT of the PV matmul.
  - a ones-column appended to v makes the PV matmul also produce the softmax
    denominator; normalization + Wv_b bias-add are fused into one DVE op.
Compute is bf16 (PE at 1 cycle/row vs 4 for fp32), accumulation fp32 in PSUM.
Softmax skips max-subtraction: scores/sqrt(D) are ~N(0, 0.65) here, bounded
by ~+-4, so exp() is safe in fp32.

Constants are packed host-side into two [128, *] DRAM tensors (Wpack/Bpack):
DMA issue cost on the sequencer is ~5ns per descriptor ~= per partition-row,
so one wide transfer beats many narrow ones.
"""

import numpy as np
import ml_dtypes

B, N, D = 4, 4096, 256
NCORES = 8
S = N // 2          # query rows per core
P = 128             # partitions
QBLK = 512          # scores free-dim block (one PSUM bank)
NJT = N // P        # 32 key tiles
NQB = S // QBLK     # 4 query blocks per core
DC = D // P         # 2 chunks of the feature dim
NWARM = 19          # PE p-state warm-up matmuls (bridge until first input DMA)

BF16 = ml_dtypes.bfloat16

_CACHE = {}


def _build():
    import concourse.mybir as mybir
    import concourse.tile as tile
    from concourse import bacc

    bf16 = mybir.dt.bfloat16
    f32 = mybir.dt.float32
    AF = mybir.ActivationFunctionType
    ALU = mybir.AluOpType

    nc = bacc.Bacc("TRN2", target_bir_lowering=False, debug=False, num_devices=NCORES)

    xT = nc.dram_tensor("xT", [DC, P, S], bf16, kind="ExternalInput").ap()
    zT = nc.dram_tensor("zT", [DC, P, N], bf16, kind="ExternalInput").ap()
    Wpack = nc.dram_tensor("Wpack", [P, 6 * D], bf16, kind="ExternalInput").ap()
    Bpack = nc.dram_tensor("Bpack", [P, 6 + D], f32, kind="ExternalInput").ap()
    out = nc.dram_tensor("out", [S, D], f32, kind="ExternalOutput").ap()

    with tile.TileContext(nc) as tc:
        with (
            tc.tile_pool(name="consts", bufs=1) as cp,
            tc.tile_pool(name="big", bufs=1) as bp,
            tc.tile_pool(name="pblk", bufs=2) as pp,
            tc.tile_pool(name="outp", bufs=4) as op,
            tc.tile_pool(name="psum", bufs=4, space="PSUM") as ps,
        ):
            # ---- PE warm-up ----
            # The PE clock ramps 0.65 -> 1.2 -> 2.4 GHz over ~4us of
            # continuous execution and resets on idle. Burn a few matmuls on
            # an uninitialized tile while the input DMAs are in flight so the
            # real matmuls start (nearly) at full clock.
            wrm = cp.tile([P, P + QBLK], bf16, tag="warm", name="warm")
            nc.vector.memset(wrm[:], 0.0)
            wps = ps.tile([P, QBLK], f32, tag="proj", name="warm_ps")
            for _ in range(NWARM):
                nc.tensor.matmul(
                    wps[:], wrm[:, 0:P], wrm[:, P:P + QBLK], start=True, stop=True
                )

            # ---- input / constant DMAs (ordered by first use) ----
            # Two HW-DGE rings issue in parallel: sync (SP) carries the
            # weights + zT, scalar (ACT) carries xT + biases. ~0.6us issue
            # cost per [128, *] transfer (one descriptor per partition row).
            wpk = cp.tile([P, 6 * D], bf16, tag="wpk", name="wpk")
            nc.sync.dma_start(wpk[:], Wpack[:])
            xT_sb, zT_sb = [], []
            for c in range(DC):
                xT_sb.append(bp.tile([P, S], bf16, tag=f"xT{c}", name=f"xT{c}"))
                zT_sb.append(bp.tile([P, N], bf16, tag=f"zT{c}", name=f"zT{c}"))
            for c in range(DC):
                nc.sync.dma_start(zT_sb[c][:], zT[c])
            for c in range(DC):
                nc.sync.dma_start(xT_sb[c][:], xT[c])
            bpk = cp.tile([P, 6 + D], f32, tag="bpk", name="bpk")
            nc.sync.dma_start(bpk[:], Bpack[:])

            def w_sl(w, c):  # lhsT [128, 256] slice for weight w, chunk c
                return wpk[:, (w * 2 + c) * D:(w * 2 + c + 1) * D]

            bq_sb = [bpk[:, c:c + 1] for c in range(DC)]
            bk_sb = [bpk[:, 2 + c:3 + c] for c in range(DC)]
            bvb_sb = bpk[:, 6:6 + D]

            # ---- kT projection: kT[e, j] over [256, 4096] ----
            kT_sb = []
            for e in range(DC):
                t = bp.tile([P, N], bf16, tag=f"kT{e}", name=f"kT{e}")
                kT_sb.append(t)
            for jb in range(N // QBLK):
                for e in range(DC):
                    acc = ps.tile([P, QBLK], f32, tag="proj", name="proj_ps")
                    for c in range(DC):
                        nc.tensor.matmul(
                            acc[:],
                            w_sl(1, c)[:, e * P:(e + 1) * P],
                            zT_sb[c][:, jb * QBLK:(jb + 1) * QBLK],
                            start=(c == 0),
                            stop=(c == DC - 1),
                        )
                    nc.scalar.activation(
                        kT_sb[e][:, jb * QBLK:(jb + 1) * QBLK], acc[:],
                        AF.Identity, bias=bk_sb[e],
                    )

            # ---- v projection: v[j, e] in 32 tiles [128, 257] (ones column) ----
            v_sb = []
            for t_i in range(NJT):
                vt = bp.tile([P, D + 1], bf16, tag=f"v{t_i}", name=f"v{t_i}")
                acc = ps.tile([P, D], f32, tag="sc", name="v_ps")
                for c in range(DC):
                    nc.tensor.matmul(
                        acc[:],
                        zT_sb[c][:, t_i * P:(t_i + 1) * P],
                        w_sl(2, c),
                        start=(c == 0),
                        stop=(c == DC - 1),
                    )
                nc.vector.tensor_copy(vt[:, 0:D], acc[:])
                nc.vector.memset(vt[:, D:D + 1], 1.0)
                v_sb.append(vt)

            # ---- qT projection: qT[e, i] over [256, 2048] ----
            qT_sb = []
            for e in range(DC):
                t = bp.tile([P, S], bf16, tag=f"qT{e}", name=f"qT{e}")
                qT_sb.append(t)
            for jb in range(S // QBLK):
                for e in range(DC):
                    acc = ps.tile([P, QBLK], f32, tag="proj", name="proj_ps")
                    for c in range(DC):
                        nc.tensor.matmul(
                            acc[:],
                            w_sl(0, c)[:, e * P:(e + 1) * P],
                            xT_sb[c][:, jb * QBLK:(jb + 1) * QBLK],
                            start=(c == 0),
                            stop=(c == DC - 1),
                        )
                    nc.scalar.activation(
                        qT_sb[e][:, jb * QBLK:(jb + 1) * QBLK], acc[:],
                        AF.Identity, bias=bq_sb[e],
                    )


            # ---- attention, per query block of 512 ----
            for qb in range(NQB):
                pts = []
                for t_i in range(NJT):
                    acc = ps.tile([P, QBLK], f32, tag="sc", name="sc_ps")
                    for c in range(DC):
                        nc.tensor.matmul(
                            acc[:],
                            kT_sb[c][:, t_i * P:(t_i + 1) * P],
                            qT_sb[c][:, qb * QBLK:(qb + 1) * QBLK],
                            start=(c == 0),
                            stop=(c == DC - 1),
                        )
                    pt = pp.tile([P, QBLK], bf16, tag=f"pT{t_i}", name=f"pT{t_i}")
                    # exp(scores / sqrt(D)); sqrt(256) = 16
                    nc.scalar.activation(pt[:], acc[:], AF.Exp, scale=1.0 / 16.0)
                    pts.append(pt)
                # PV accumulation with t outer so PE consumes exp'd tiles in
                # the order ACT produces them (no stall on the last exps).
                pvs = [
                    ps.tile([P, D + 1], f32, tag="proj", name=f"pv_ps{sq}")
                    for sq in range(QBLK // P)
                ]
                for t_i in range(NJT):
                    for sq in range(QBLK // P):
                        nc.tensor.matmul(
                            pvs[sq][:],
                            pts[t_i][:, sq * P:(sq + 1) * P],
                            v_sb[t_i][:],
                            start=(t_i == 0),
                            stop=(t_i == NJT - 1),
                        )
                for sq in range(QBLK // P):
                    pv = pvs[sq]
                    recip = op.tile([P, 1], f32, tag="recip", name="recip")
                    nc.vector.reciprocal(recip[:], pv[:, D:D + 1])
                    ot = op.tile([P, D], f32, tag="ot", name="ot")
                    # out = (pv * 1/denom) + bv
                    nc.vector.scalar_tensor_tensor(
                        ot[:], pv[:, 0:D], recip[:], bvb_sb,
                        op0=ALU.mult, op1=ALU.add,
                    )
                    r0 = (qb * (QBLK // P) + sq) * P
                    nc.sync.dma_start(out[r0:r0 + P, :], ot[:])

    nc.compile()
    return nc


def _get_nc():
    if "nc" not in _CACHE:
        _CACHE["nc"] = _build()
    return _CACHE["nc"]


def _prep_in_maps(x, z, Wq_w, Wq_b, Wk_w, Wk_b, Wv_w, Wv_b):
    x = np.asarray(x, np.float32)
    z = np.asarray(z, np.float32)

    Wpack = np.empty((P, 6 * D), BF16)
    for w, W in enumerate((Wq_w, Wk_w, Wv_w)):
        WT = np.ascontiguousarray(np.asarray(W, np.float32).T).astype(BF16)
        for c in range(DC):
            Wpack[:, (w * 2 + c) * D:(w * 2 + c + 1) * D] = WT[c * P:(c + 1) * P, :]
    Bpack = np.zeros((P, 6 + D), np.float32)
    for c in range(DC):
        Bpack[:, c] = np.asarray(Wq_b, np.float32)[c * P:(c + 1) * P]
        Bpack[:, 2 + c] = np.asarray(Wk_b, np.float32)[c * P:(c + 1) * P]
    Bpack[:, 6:] = np.broadcast_to(np.asarray(Wv_b, np.float32), (P, D))

    in_maps = []
    for core in range(NCORES):
        b, h = divmod(core, 2)
        xTc = np.ascontiguousarray(x[b].T[:, h * S:(h + 1) * S]).astype(BF16)
        zTc = np.ascontiguousarray(z[b].T).astype(BF16)
        in_maps.append({
            "xT": xTc.reshape(DC, P, S),
            "zT": zTc.reshape(DC, P, N),
            "Wpack": Wpack, "Bpack": Bpack,
        })
    return in_maps


def kernel(x, z, Wq_w, Wq_b, Wk_w, Wk_b, Wv_w, Wv_b):
    from concourse.bass_utils import run_bass_kernel_spmd

    in_maps = _prep_in_maps(x, z, Wq_w, Wq_b, Wk_w, Wk_b, Wv_w, Wv_b)
    nc = _get_nc()
    _CACHE["in_maps"] = in_maps
    res = run_bass_kernel_spmd(nc, in_maps, core_ids=list(range(NCORES)))

    full = np.empty((B, N, D), np.float32)
    for core in range(NCORES):
        b, h = divmod(core, 2)
        full[b, h * S:(h + 1) * S, :] = res.results[core]["out"]
    return full
